# revision 6
# baseline (speedup 1.0000x reference)
"""Bass/Tile TRN2 kernel for nn_CropDrones.

Op: per-sample, find bbox of a binary window mask (channel 3 of input1),
crop rows [r0, r1) x cols [c0, c1) of the 3 image channels, and paste the
crop centered into a 256x256 zero canvas.

Sharding: pure data parallel - batch 32 split as 8 cores x 4 samples.

Device strategy (per sample, all control flow static; data dependence
flows only through values):
  1. Load the 512x512 mask as [128p, 4n, 512w]. Row sums via 4 Activation
     Identity+accum ops (srow[p,n] = sum over w; equals sw+1 inside the
     window, 0 outside). Col stats on DVE: tp2/colp pairwise maxes give
     colp[p,w] = colany[w]*rowanyp[p]; the producing scalar_tensor_tensor
     emits accum_out = s~ = (sw+1)*rowanyp free of charge, and one more
     st against iota_w gives the first moment m~ = sum(w*colany)*rowanyp.
  2. One gpsimd partition_all_reduce(max) over [r1cand, r0revcand, s~, m~]
     broadcasts exact bbox stats to all partitions (r candidates from the
     0/1 rowany weighted-max trick; s~/m~ scale with rowanyp so the max
     over partitions is the clean value).
  3. c0+c1 = 2*m/s recovered exactly via DVE reciprocal + f32->i32
     round-to-nearest (error ~1e-4 << 0.5); top/left via i32 shift-right.
  4. Row gather fused with the fine column shift: per channel c, ONE
     element-granular indirect DMA (src viewed as [N,1], coef=1): index
     of partition p = base + clamp(rt+2p,0,511)*512 + (c0-left), each
     copying 768 contiguous floats = output rows (2p, 2p+1) at offsets
     0..256 and 512..768. No ap_gather, no full-row overfetch.
  5. Masks: mx (col validity) on DVE, mk[k] = my[k]*mx built on the
     Activation engine; fin = gat*mk via 3 tensor_tensor ops; one DMA
     stores [128, (c k x)] as y[i, c, 2p+k, x].

Verified bit-exact vs the jax reference (relative error 0.0).
"""

import numpy as np

import concourse.bass as bass
import concourse.bacc as bacc
import concourse.bass_isa as bass_isa
import concourse.mybir as mybir
from concourse.bass import IndirectOffsetOnAxis
from concourse.bass_utils import run_bass_kernel_spmd
from concourse.tile import TileContext

# Problem shapes (hardcoded; kernel.py must be self-contained).
B, C, H, W = 32, 3, 512, 512
CH_IN = 4          # image channels + mask channel
S = 256            # output side
N_CORES = 8
BPC = B // N_CORES  # samples per core
P = 128
NT = H // P        # mask row tiles
NEL = BPC * CH_IN * H * W  # flat element count of x
GL = 2 * W - S     # gathered run per index: rows (2p, 2p+1) -> 768 floats

f32 = mybir.dt.float32
i32 = mybir.dt.int32
Alu = mybir.AluOpType
Ax = mybir.AxisListType
Act = mybir.ActivationFunctionType


_C_WIDTHS = {
    "c_iota_w": W,     # [128, 512] 0..511 per row
    "c_x256": S,       # [128, 256] 0..255
    "c_iota_r": NT,    # [128, 4] r = n*128+p
    "c_rev_r": NT,     # 511 - r
    "c_y2": 2,         # output rows per partition: 2p, 2p+1
    "c_2p": 1,         # 2p
    "c_coff": C,       # c * H * W
}
_C_OFFS = {}
_off = 0
for _k, _w in _C_WIDTHS.items():
    _C_OFFS[_k] = _off
    _off += _w
C_TOTAL = _off


def _consts() -> dict[str, np.ndarray]:
    p = np.arange(P)
    iota_w = np.broadcast_to(np.arange(W, dtype=np.float32), (P, W)).copy()
    x256 = np.broadcast_to(np.arange(S, dtype=np.float32), (P, S)).copy()
    iota_r = (p[:, None] + P * np.arange(NT)[None, :]).astype(np.float32)
    rev_r = (H - 1.0) - iota_r
    y2 = (2 * p[:, None] + np.arange(2)[None, :]).astype(np.float32)
    c2p = (2.0 * p[:, None]).astype(np.float32)
    coff = np.broadcast_to(
        (np.arange(C) * H * W).astype(np.float32), (P, C)
    ).copy()
    vals = {
        "c_iota_w": iota_w,
        "c_x256": x256,
        "c_iota_r": iota_r,
        "c_rev_r": rev_r,
        "c_y2": y2,
        "c_2p": c2p,
        "c_coff": coff,
    }
    packed = np.zeros((P, C_TOTAL), dtype=np.float32)
    for kk, vv in vals.items():
        packed[:, _C_OFFS[kk] : _C_OFFS[kk] + _C_WIDTHS[kk]] = vv
    return {"c_all": packed}


def _build() -> bass.Bass:
    nc = bacc.Bacc("TRN2")
    x = nc.dram_tensor("x", [BPC, CH_IN, H, W], f32, kind="ExternalInput")
    y = nc.dram_tensor("y", [BPC, C, S, S], f32, kind="ExternalOutput")
    c_all = nc.dram_tensor("c_all", [P, C_TOTAL], f32, kind="ExternalInput")

    with TileContext(nc) as tc:
        with (
            tc.tile_pool(name="consts", bufs=1) as cpool,
            tc.tile_pool(name="work", bufs=4) as wp,
        ):
            call_t = cpool.tile([P, C_TOTAL], f32, tag="c_all", name="c_all_t")
            nc.sync.dma_start(call_t[:], c_all[:])
            ct = {
                k: call_t[:, _C_OFFS[k] : _C_OFFS[k] + _C_WIDTHS[k]]
                for k in _C_WIDTHS
            }

            ts = nc.vector.tensor_scalar
            tsp = nc.gpsimd.tensor_scalar
            st = nc.vector.scalar_tensor_tensor
            tt = nc.vector.tensor_tensor
            ttp = nc.gpsimd.tensor_tensor
            red = nc.vector.tensor_reduce
            cpy = nc.vector.tensor_copy
            cpyp = nc.gpsimd.tensor_copy
            act = nc.scalar.activation

            for i in range(BPC):
                # ---- 1. mask load ----
                mt = wp.tile([P, NT, W], f32, tag="mt", name=f"mt{i}")
                nc.sync.dma_start(
                    mt[:], x[i, CH_IN - 1].rearrange("(n p) w -> p n w", p=P)
                )

                # ---- 2. row sums on Activation (accum), col stats on DVE ----
                junka = wp.tile([P, W], f32, tag="junka", name=f"ja{i}")
                srow = wp.tile([P, NT], f32, tag="srow", name=f"srow{i}")
                for n in range(NT):
                    act(junka[:], mt[:, n, :], Act.Identity, scale=1.0,
                        accum_out=srow[:, n : n + 1])

                cand = wp.tile([P, 4], f32, tag="cand", name=f"cand{i}")
                tp2 = wp.tile([P, 2, W], f32, tag="tp2", name=f"tp2{i}")
                colp = wp.tile([P, W], f32, tag="colp", name=f"colp{i}")
                junkd = wp.tile([P, W], f32, tag="junkd", name=f"jd{i}")
                tt(tp2[:], mt[:, 0:2, :], mt[:, 2:4, :], op=Alu.max)
                # colp = max(tp2[0], tp2[1]); accum -> s~ = (sw+1)*rowanyp
                st(colp[:], tp2[:, 0, :], 1.0, tp2[:, 1, :],
                   op0=Alu.mult, op1=Alu.max, accum_out=cand[:, 2:3])
                # m~ = sum(w * colp)
                st(junkd[:], colp[:], 1.0, ct["c_iota_w"],
                   op0=Alu.mult, op1=Alu.mult, accum_out=cand[:, 3:4])

                # rows: 0/1 any + weighted max (exact ints, no division)
                ra = wp.tile([P, NT], f32, tag="ra", name=f"ra{i}")
                t4 = wp.tile([P, NT], f32, tag="t4", name=f"t4{i}")
                t4b = wp.tile([P, NT], f32, tag="t4b", name=f"t4b{i}")
                ts(ra[:], srow[:], 0.0, None, op0=Alu.is_gt)
                tt(t4[:], ra[:], ct["c_iota_r"], op=Alu.mult)
                red(cand[:, 0:1], t4[:], axis=Ax.X, op=Alu.max)   # r1
                tt(t4b[:], ra[:], ct["c_rev_r"], op=Alu.mult)
                red(cand[:, 1:2], t4b[:], axis=Ax.X, op=Alu.max)  # 511-r0

                # ---- 3. broadcast bbox stats ----
                ar = wp.tile([P, 4], f32, tag="ar", name=f"ar{i}")
                nc.gpsimd.partition_all_reduce(
                    ar[:], cand[:], channels=P, reduce_op=bass_isa.ReduceOp.max
                )

                # ---- 4. scalar chain ----
                # sc: 0 rs, 1 q(2mu), 2 kf, 3 2c0, 4 c0, 5 sw, 6 256-sw,
                #     7 left, 8 d, 9 lw, 10 r0, 11 sh
                # sc2: 0 256-sh, 1 top, 2 rt, 3 tph
                sc = wp.tile([P, 12], f32, tag="sc", name=f"sc{i}")
                sc2 = wp.tile([P, 4], f32, tag="sc2", name=f"sc2{i}")
                sci = wp.tile([P, 4], i32, tag="sci", name=f"sci{i}")

                # cols subchain (DVE for recip/cvt, Pool for the rest)
                nc.vector.reciprocal(sc[:, 0:1], ar[:, 2:3])
                ts(sc[:, 1:2], ar[:, 3:4], sc[:, 0:1], 2.0,
                   op0=Alu.mult, op1=Alu.mult)            # 2*mu_c approx
                cpy(sci[:, 0:1], sc[:, 1:2])              # round -> c0+c1
                cpy(sc[:, 2:3], sci[:, 0:1])
                ts(sc[:, 3:4], sc[:, 2:3], ar[:, 2:3], 1.0,
                   op0=Alu.subtract, op1=Alu.add)         # 2c0 = k - s + 1
                ts(sc[:, 4:5], sc[:, 3:4], 0.5, None, op0=Alu.mult)  # c0
                ts(sc[:, 5:6], ar[:, 2:3], 1.0, None, op0=Alu.subtract)  # sw
                ts(sc[:, 6:7], sc[:, 5:6], -1.0, 256.0,
                    op0=Alu.mult, op1=Alu.add)            # 256-sw
                cpy(sci[:, 1:2], sc[:, 6:7])
                ts(sci[:, 1:2], sci[:, 1:2], 1, None,
                    op0=Alu.logical_shift_right)
                cpy(sc[:, 7:8], sci[:, 1:2])             # left
                tt(sc[:, 8:9], sc[:, 4:5], sc[:, 7:8], op=Alu.subtract)  # d
                tt(sc[:, 9:10], sc[:, 7:8], sc[:, 5:6], op=Alu.add)      # lw

                # rows subchain (DVE)
                ts(sc[:, 10:11], ar[:, 1:2], -1.0, 511.0,
                   op0=Alu.mult, op1=Alu.add)             # r0
                tt(sc[:, 11:12], ar[:, 0:1], sc[:, 10:11], op=Alu.subtract)  # sh
                ts(sc2[:, 0:1], sc[:, 11:12], -1.0, 256.0,
                   op0=Alu.mult, op1=Alu.add)             # 256-sh
                cpy(sci[:, 2:3], sc2[:, 0:1])
                ts(sci[:, 2:3], sci[:, 2:3], 1, None, op0=Alu.logical_shift_right)
                cpy(sc2[:, 1:2], sci[:, 2:3])             # top
                tt(sc2[:, 2:3], sc[:, 10:11], sc2[:, 1:2], op=Alu.subtract)  # rt
                tt(sc2[:, 3:4], sc2[:, 1:2], sc[:, 11:12], op=Alu.add)       # tph

                # ---- 5. gather indices + 3 indirect DMAs ----
                iy = wp.tile([P, 1], f32, tag="iy", name=f"iy{i}")
                ix = wp.tile([P, C], f32, tag="ix", name=f"ix{i}")
                ri = wp.tile([P, C], i32, tag="ri", name=f"ri{i}")
                ts(iy[:], ct["c_2p"], sc2[:, 2:3], None, op0=Alu.add)
                ts(iy[:], iy[:], 0.0, float(H - 1), op0=Alu.max, op1=Alu.min)
                ts(iy[:], iy[:], float(W), None, op0=Alu.mult)
                ts(ix[:], ct["c_coff"], iy[:], float(i * CH_IN * H * W),
                    op0=Alu.add, op1=Alu.add)             # coff + y*512 + base
                ts(ix[:], ix[:], sc[:, 8:9], 0.0,
                    op0=Alu.add, op1=Alu.max)             # + d, clamp >= 0
                cpy(ri[:], ix[:])

                gat = wp.tile([P, C, GL], f32, tag="gat", name=f"gat{i}")
                for c in range(C):
                    nc.gpsimd.indirect_dma_start(
                        out=gat[:, c, :],
                        out_offset=None,
                        in_=bass.AP(x[:].tensor, 0, [[1, NEL], [1, 1]]),
                        in_offset=IndirectOffsetOnAxis(ap=ri[:, c : c + 1], axis=0),
                    )

                # ---- 6. masks ----
                mya = wp.tile([P, 2], f32, tag="mya", name=f"mya{i}")
                myb = wp.tile([P, 2], f32, tag="myb", name=f"myb{i}")
                ts(mya[:], ct["c_y2"], sc2[:, 1:2], None, op0=Alu.is_ge)
                ts(myb[:], ct["c_y2"], sc2[:, 3:4], None, op0=Alu.is_lt)
                tt(mya[:], mya[:], myb[:], op=Alu.mult)   # my [128,2]
                mxa = wp.tile([P, S], f32, tag="mxa", name=f"mxa{i}")
                mxb = wp.tile([P, S], f32, tag="mxb", name=f"mxb{i}")
                ts(mxa[:], ct["c_x256"], sc[:, 7:8], None, op0=Alu.is_ge)
                ts(mxb[:], ct["c_x256"], sc[:, 9:10], None, op0=Alu.is_lt)
                tt(mxa[:], mxa[:], mxb[:], op=Alu.mult)   # mx [128,256]
                mk = wp.tile([P, 2, S], f32, tag="mk", name=f"mk{i}")
                for k in range(2):
                    act(mk[:, k, :], mxa[:], Act.Identity,
                        scale=mya[:, k : k + 1])

                # ---- 7. apply masks + store ----
                fin = wp.tile([P, C, 2, S], f32, tag="fin", name=f"fin{i}")
                for c in range(C):
                    g = gat[:, c, :]
                    gv = bass.AP(g.tensor, g.offset,
                                 [list(g.ap[0]), [W, 2], [1, S]])
                    tt(fin[:, c], gv, mk[:], op=Alu.mult)
                # y[i, c, 2p+k, x] <- fin[p, c, k, x]; (k x) merge to 512
                yv = y[i]
                ydst = bass.AP(yv.tensor, yv.offset,
                               [[2 * S, P], [S * S, C], [1, 2 * S]])
                nc.sync.dma_start(ydst, fin[:])
    nc.finalize()
    return nc


_CACHE: dict[str, object] = {}


def kernel(input1: np.ndarray, input2: np.ndarray, **_: np.ndarray) -> np.ndarray:
    input1 = np.ascontiguousarray(np.asarray(input1, dtype=np.float32))
    if "nc" not in _CACHE:
        _CACHE["nc"] = _build()
        _CACHE["consts"] = _consts()
    nc = _CACHE["nc"]
    consts = _CACHE["consts"]
    in_maps = [
        {"x": np.ascontiguousarray(input1[k * BPC : (k + 1) * BPC]), **consts}
        for k in range(N_CORES)
    ]
    res = run_bass_kernel_spmd(nc, in_maps, core_ids=list(range(N_CORES)))
    out = np.concatenate([r["y"] for r in res.results], axis=0)
    return out.astype(np.float32)


if __name__ == "__main__":
    rng = np.random.default_rng(1)
    x = rng.standard_normal((B, CH_IN, H, W), dtype=np.float32)
    print(kernel(x, np.zeros((B, C, S, S), np.float32)).shape)


# revision 7
# speedup vs baseline: 12.8454x; 12.8454x over previous
"""Bass/Tile TRN2 kernel for nn_CropDrones.

Op: per-sample, find bbox of a binary window mask (channel 3 of input1),
crop rows [r0, r1) x cols [c0, c1) of the 3 image channels, and paste the
crop centered into a 256x256 zero canvas.

Sharding: pure data parallel - batch 32 split as 8 cores x 4 samples.

Device strategy (per sample, all control flow static; data dependence
flows only through values):
  1. Load the 512x512 mask as [128p, 4n, 512w]. Row sums via 4 Activation
     Identity+accum ops (srow[p,n] = sum over w; equals sw+1 inside the
     window, 0 outside). Col stats on DVE: tp2/colp pairwise maxes give
     colp[p,w] = colany[w]*rowanyp[p]; the producing scalar_tensor_tensor
     emits accum_out = s~ = (sw+1)*rowanyp free of charge, and one more
     st against iota_w gives the first moment m~ = sum(w*colany)*rowanyp.
  2. One gpsimd partition_all_reduce(max) over [r1cand, r0revcand, s~, m~]
     broadcasts exact bbox stats to all partitions (r candidates from the
     0/1 rowany weighted-max trick; s~/m~ scale with rowanyp so the max
     over partitions is the clean value).
  3. c0+c1 = 2*m/s recovered exactly via DVE reciprocal + f32->i32
     round-to-nearest (error ~1e-4 << 0.5); top/left via i32 shift-right.
  4. Row gather fused with the fine column shift: per channel c, ONE
     element-granular indirect DMA (src viewed as [N,1], coef=1): index
     of partition p = base + clamp(rt+2p,0,511)*512 + (c0-left), each
     copying 768 contiguous floats = output rows (2p, 2p+1) at offsets
     0..256 and 512..768. No ap_gather, no full-row overfetch.
  5. Masks: mx (col validity) on DVE, mk[k] = my[k]*mx built on the
     Activation engine; fin = gat*mk via 3 tensor_tensor ops; one DMA
     stores [128, (c k x)] as y[i, c, 2p+k, x].

Verified bit-exact vs the jax reference (relative error 0.0).
"""

import numpy as np

import concourse.bass as bass
import concourse.bacc as bacc
import concourse.bass_isa as bass_isa
import concourse.mybir as mybir
from concourse.bass import IndirectOffsetOnAxis
from concourse.bass_utils import run_bass_kernel_spmd
from concourse.tile import TileContext

# Problem shapes (hardcoded; kernel.py must be self-contained).
B, C, H, W = 32, 3, 512, 512
CH_IN = 4          # image channels + mask channel
S = 256            # output side
N_CORES = 8
BPC = B // N_CORES  # samples per core
P = 128
NT = H // P        # mask row tiles
NEL = BPC * CH_IN * H * W  # flat element count of x
GL = 2 * W - S     # gathered run per index: rows (2p, 2p+1) -> 768 floats

f32 = mybir.dt.float32
i32 = mybir.dt.int32
Alu = mybir.AluOpType
Ax = mybir.AxisListType
Act = mybir.ActivationFunctionType


_C_WIDTHS = {
    "c_iota_w": W,     # [128, 512] 0..511 per row
    "c_x256": S,       # [128, 256] 0..255
    "c_iota_r": NT,    # [128, 4] r = n*128+p
    "c_rev_r": NT,     # 511 - r
    "c_y2": 2,         # output rows per partition: 2p, 2p+1
    "c_2p": 1,         # 2p
    "c_coff": C,       # c * H * W
}
_C_OFFS = {}
_off = 0
for _k, _w in _C_WIDTHS.items():
    _C_OFFS[_k] = _off
    _off += _w
C_TOTAL = _off


def _consts() -> dict[str, np.ndarray]:
    p = np.arange(P)
    iota_w = np.broadcast_to(np.arange(W, dtype=np.float32), (P, W)).copy()
    x256 = np.broadcast_to(np.arange(S, dtype=np.float32), (P, S)).copy()
    iota_r = (p[:, None] + P * np.arange(NT)[None, :]).astype(np.float32)
    rev_r = (H - 1.0) - iota_r
    y2 = (2 * p[:, None] + np.arange(2)[None, :]).astype(np.float32)
    c2p = (2.0 * p[:, None]).astype(np.float32)
    coff = np.broadcast_to(
        (np.arange(C) * H * W).astype(np.float32), (P, C)
    ).copy()
    vals = {
        "c_iota_w": iota_w,
        "c_x256": x256,
        "c_iota_r": iota_r,
        "c_rev_r": rev_r,
        "c_y2": y2,
        "c_2p": c2p,
        "c_coff": coff,
    }
    packed = np.zeros((P, C_TOTAL), dtype=np.float32)
    for kk, vv in vals.items():
        packed[:, _C_OFFS[kk] : _C_OFFS[kk] + _C_WIDTHS[kk]] = vv
    return {"c_all": packed}


def _build() -> bass.Bass:
    nc = bacc.Bacc("TRN2")
    x = nc.dram_tensor("x", [BPC, CH_IN, H, W], f32, kind="ExternalInput")
    y = nc.dram_tensor("y", [BPC, C, S, S], f32, kind="ExternalOutput")
    c_all = nc.dram_tensor("c_all", [P, C_TOTAL], f32, kind="ExternalInput")

    with TileContext(nc) as tc:
        with (
            tc.tile_pool(name="consts", bufs=1) as cpool,
            tc.tile_pool(name="work", bufs=4) as wp,
        ):
            call_t = cpool.tile([P, C_TOTAL], f32, tag="c_all", name="c_all_t")
            nc.sync.dma_start(call_t[:], c_all[:])
            ct = {
                k: call_t[:, _C_OFFS[k] : _C_OFFS[k] + _C_WIDTHS[k]]
                for k in _C_WIDTHS
            }

            ts = nc.vector.tensor_scalar
            tsp = nc.gpsimd.tensor_scalar
            st = nc.vector.scalar_tensor_tensor
            tt = nc.vector.tensor_tensor
            ttp = nc.gpsimd.tensor_tensor
            red = nc.vector.tensor_reduce
            cpy = nc.vector.tensor_copy
            cpyp = nc.gpsimd.tensor_copy
            act = nc.scalar.activation

            for i in range(BPC):
                # ---- 1. mask load ----
                mt = wp.tile([P, NT, W], f32, tag="mt", name=f"mt{i}")
                nc.sync.dma_start(
                    mt[:], x[i, CH_IN - 1].rearrange("(n p) w -> p n w", p=P)
                )

                # ---- 2. row sums on Activation (accum), col stats on DVE ----
                junka = wp.tile([P, W], f32, tag="junka", name=f"ja{i}")
                srow = wp.tile([P, NT], f32, tag="srow", name=f"srow{i}")
                for n in range(NT):
                    act(junka[:], mt[:, n, :], Act.Identity, scale=1.0,
                        accum_out=srow[:, n : n + 1])

                cand = wp.tile([P, 4], f32, tag="cand", name=f"cand{i}")
                tp2 = wp.tile([P, 2, W], f32, tag="tp2", name=f"tp2{i}")
                colp = wp.tile([P, W], f32, tag="colp", name=f"colp{i}")
                junkd = wp.tile([P, W], f32, tag="junkd", name=f"jd{i}")
                tt(tp2[:], mt[:, 0:2, :], mt[:, 2:4, :], op=Alu.max)
                # colp = max(tp2[0], tp2[1]); accum -> s~ = (sw+1)*rowanyp
                st(colp[:], tp2[:, 0, :], 1.0, tp2[:, 1, :],
                   op0=Alu.mult, op1=Alu.max, accum_out=cand[:, 2:3])
                # m~ = sum(w * colp)
                st(junkd[:], colp[:], 1.0, ct["c_iota_w"],
                   op0=Alu.mult, op1=Alu.mult, accum_out=cand[:, 3:4])

                # rows: 0/1 any + weighted max (exact ints, no division)
                ra = wp.tile([P, NT], f32, tag="ra", name=f"ra{i}")
                t4 = wp.tile([P, NT], f32, tag="t4", name=f"t4{i}")
                t4b = wp.tile([P, NT], f32, tag="t4b", name=f"t4b{i}")
                ts(ra[:], srow[:], 0.0, None, op0=Alu.is_gt)
                tt(t4[:], ra[:], ct["c_iota_r"], op=Alu.mult)
                red(cand[:, 0:1], t4[:], axis=Ax.X, op=Alu.max)   # r1
                tt(t4b[:], ra[:], ct["c_rev_r"], op=Alu.mult)
                red(cand[:, 1:2], t4b[:], axis=Ax.X, op=Alu.max)  # 511-r0

                # ---- 3. broadcast bbox stats ----
                ar = wp.tile([P, 4], f32, tag="ar", name=f"ar{i}")
                nc.gpsimd.partition_all_reduce(
                    ar[:], cand[:], channels=P, reduce_op=bass_isa.ReduceOp.max
                )

                # ---- 4. scalar chain ----
                # sc: 0 rs, 1 q(2mu), 2 kf, 3 2c0, 4 c0, 5 sw, 6 256-sw,
                #     7 left, 8 d, 9 lw, 10 r0, 11 sh
                # sc2: 0 256-sh, 1 top, 2 rt, 3 tph
                sc = wp.tile([P, 12], f32, tag="sc", name=f"sc{i}")
                sc2 = wp.tile([P, 4], f32, tag="sc2", name=f"sc2{i}")
                sci = wp.tile([P, 4], i32, tag="sci", name=f"sci{i}")

                # cols subchain (DVE for recip/cvt, Pool for the rest)
                nc.vector.reciprocal(sc[:, 0:1], ar[:, 2:3])
                ts(sc[:, 1:2], ar[:, 3:4], sc[:, 0:1], 2.0,
                   op0=Alu.mult, op1=Alu.mult)            # 2*mu_c approx
                cpy(sci[:, 0:1], sc[:, 1:2])              # round -> c0+c1
                cpy(sc[:, 2:3], sci[:, 0:1])
                ts(sc[:, 3:4], sc[:, 2:3], ar[:, 2:3], 1.0,
                   op0=Alu.subtract, op1=Alu.add)         # 2c0 = k - s + 1
                ts(sc[:, 4:5], sc[:, 3:4], 0.5, None, op0=Alu.mult)  # c0
                ts(sc[:, 5:6], ar[:, 2:3], 1.0, None, op0=Alu.subtract)  # sw
                ts(sc[:, 6:7], sc[:, 5:6], -1.0, 256.0,
                    op0=Alu.mult, op1=Alu.add)            # 256-sw
                cpy(sci[:, 1:2], sc[:, 6:7])
                ts(sci[:, 1:2], sci[:, 1:2], 1, None,
                    op0=Alu.logical_shift_right)
                cpy(sc[:, 7:8], sci[:, 1:2])             # left
                tt(sc[:, 8:9], sc[:, 4:5], sc[:, 7:8], op=Alu.subtract)  # d
                tt(sc[:, 9:10], sc[:, 7:8], sc[:, 5:6], op=Alu.add)      # lw

                # rows subchain (DVE)
                ts(sc[:, 10:11], ar[:, 1:2], -1.0, 511.0,
                   op0=Alu.mult, op1=Alu.add)             # r0
                tt(sc[:, 11:12], ar[:, 0:1], sc[:, 10:11], op=Alu.subtract)  # sh
                ts(sc2[:, 0:1], sc[:, 11:12], -1.0, 256.0,
                   op0=Alu.mult, op1=Alu.add)             # 256-sh
                cpy(sci[:, 2:3], sc2[:, 0:1])
                ts(sci[:, 2:3], sci[:, 2:3], 1, None, op0=Alu.logical_shift_right)
                cpy(sc2[:, 1:2], sci[:, 2:3])             # top
                tt(sc2[:, 2:3], sc[:, 10:11], sc2[:, 1:2], op=Alu.subtract)  # rt
                tt(sc2[:, 3:4], sc2[:, 1:2], sc[:, 11:12], op=Alu.add)       # tph

                # ---- 5. gather indices + 3 indirect DMAs ----
                iy = wp.tile([P, 1], f32, tag="iy", name=f"iy{i}")
                ix = wp.tile([P, C], f32, tag="ix", name=f"ix{i}")
                ri = wp.tile([P, C], i32, tag="ri", name=f"ri{i}")
                ts(iy[:], ct["c_2p"], sc2[:, 2:3], None, op0=Alu.add)
                ts(iy[:], iy[:], 0.0, float(H - 1), op0=Alu.max, op1=Alu.min)
                ts(iy[:], iy[:], float(W), None, op0=Alu.mult)
                ts(ix[:], ct["c_coff"], iy[:], float(i * CH_IN * H * W),
                    op0=Alu.add, op1=Alu.add)             # coff + y*512 + base
                ts(ix[:], ix[:], sc[:, 8:9], 0.0,
                    op0=Alu.add, op1=Alu.max)             # + d, clamp >= 0
                cpy(ri[:], ix[:])

                gat = wp.tile([P, C, GL], f32, tag="gat", name=f"gat{i}")
                for c in range(C):
                    # src viewed as overlapping GL-wide rows so descgen emits
                    # 128 x 3KB descriptors; coef patched to 1 for
                    # element-granular starts (row r, col d in one index).
                    binst = nc.gpsimd.indirect_dma_start(
                        out=gat[:, c, :],
                        out_offset=None,
                        in_=bass.AP(x[:].tensor, 0,
                                    [[1, NEL - GL + 1], [1, GL]]),
                        in_offset=IndirectOffsetOnAxis(ap=ri[:, c : c + 1], axis=0),
                    )
                    a0 = binst.ins.ins[0]
                    d0 = a0.dynamic_ap_info
                    a0.dynamic_ap_info = mybir.DynamicAccessPatternInfo(
                        c=d0.c, actual_ap=d0.actual_ap,
                        indirect_dim_max_index=d0.indirect_dim_max_index,
                        offset_expr=[mybir.DynamicAccessPatternOffsetExpr(
                            coef=1, aff_expr=d0.offset_expr[0].aff_expr)])

                # ---- 6. masks ----
                mya = wp.tile([P, 2], f32, tag="mya", name=f"mya{i}")
                myb = wp.tile([P, 2], f32, tag="myb", name=f"myb{i}")
                ts(mya[:], ct["c_y2"], sc2[:, 1:2], None, op0=Alu.is_ge)
                ts(myb[:], ct["c_y2"], sc2[:, 3:4], None, op0=Alu.is_lt)
                tt(mya[:], mya[:], myb[:], op=Alu.mult)   # my [128,2]
                mxa = wp.tile([P, S], f32, tag="mxa", name=f"mxa{i}")
                mxb = wp.tile([P, S], f32, tag="mxb", name=f"mxb{i}")
                ts(mxa[:], ct["c_x256"], sc[:, 7:8], None, op0=Alu.is_ge)
                ts(mxb[:], ct["c_x256"], sc[:, 9:10], None, op0=Alu.is_lt)
                tt(mxa[:], mxa[:], mxb[:], op=Alu.mult)   # mx [128,256]
                mk = wp.tile([P, 2, S], f32, tag="mk", name=f"mk{i}")
                for k in range(2):
                    act(mk[:, k, :], mxa[:], Act.Identity,
                        scale=mya[:, k : k + 1])

                # ---- 7. apply masks + store ----
                fin = wp.tile([P, C, 2, S], f32, tag="fin", name=f"fin{i}")
                for c in range(C):
                    g = gat[:, c, :]
                    gv = bass.AP(g.tensor, g.offset,
                                 [list(g.ap[0]), [W, 2], [1, S]])
                    tt(fin[:, c], gv, mk[:], op=Alu.mult)
                # y[i, c, 2p+k, x] <- fin[p, c, k, x]; (k x) merge to 512
                yv = y[i]
                ydst = bass.AP(yv.tensor, yv.offset,
                               [[2 * S, P], [S * S, C], [1, 2 * S]])
                nc.sync.dma_start(ydst, fin[:])
    nc.finalize()
    return nc


_CACHE: dict[str, object] = {}


def kernel(input1: np.ndarray, input2: np.ndarray, **_: np.ndarray) -> np.ndarray:
    input1 = np.ascontiguousarray(np.asarray(input1, dtype=np.float32))
    if "nc" not in _CACHE:
        _CACHE["nc"] = _build()
        _CACHE["consts"] = _consts()
    nc = _CACHE["nc"]
    consts = _CACHE["consts"]
    in_maps = [
        {"x": np.ascontiguousarray(input1[k * BPC : (k + 1) * BPC]), **consts}
        for k in range(N_CORES)
    ]
    res = run_bass_kernel_spmd(nc, in_maps, core_ids=list(range(N_CORES)))
    out = np.concatenate([r["y"] for r in res.results], axis=0)
    return out.astype(np.float32)


if __name__ == "__main__":
    rng = np.random.default_rng(1)
    x = rng.standard_normal((B, CH_IN, H, W), dtype=np.float32)
    print(kernel(x, np.zeros((B, C, S, S), np.float32)).shape)
